# revision 1
# baseline (speedup 1.0000x reference)
"""GRU decoder kernel for Trainium2 (Bass/Tile), SPMD over 8 NeuronCores.

Problem: B=64, H=256, T=2000 GRU recurrence + output projection to 128 dims.
  gi = z @ Wih.T + bih            (precomputed on host: tiny, one-time)
  loop t: gh = h @ Whh.T + bhh; r,zg = sigmoid; n = tanh(i_n + r*h_n)
          h = (1-zg)*n + zg*h
  out = hs @ Wout.T + bout        -> (64, 2000, 128) fp32

Sharding: data-parallel over batch, 8 batch rows per core, weights replicated.

Layout is "gate-major": gate/hidden dims on SBUF partitions, batch on the free
dim.  The recurrent matmul keeps Whh.T tiles as the PE stationary operand
(12 tiles of 128x128, fp16 so FWL halves LDWEIGHTS); the moving operand is a
fp16 cast of h (the fp32 master state is carried in SBUF, so only the damped
matmul path sees fp16 — measured end-to-end error ~1.5e-4).  The constant
i-gates + bhh bias is injected directly into PSUM with an identity-rhs matmul
so the sigmoid reads PSUM directly.  The three gates use separate PSUM banks
ordered r, n, z so the r-sigmoid and the tanh path start before the sweep
finishes.  Hidden states are stored fp32 in SBUF (128KB/partition ring over
all 2000 steps) and consumed in place as the projection's stationary operand.
"""

import sys

sys.path.insert(0, "/opt/trn_rl_repo")

import numpy as np
import ml_dtypes
from contextlib import ExitStack

import concourse.bass as bass
import concourse.tile as tile
from concourse import bacc, mybir
from concourse import bass_utils

F32 = mybir.dt.float32
BF16 = mybir.dt.float16
AF = mybir.ActivationFunctionType

H = 256
B = 64
NCORES = 8
BL = B // NCORES  # 8 batch rows per core
OUT_D = 128
PROJ_CHUNK = 16  # timesteps per projection matmul (16*8 batch = 128 = M)

# gate order within the sweep: r first (feeds sigmoid early), n second
# (feeds the tanh chain), z last (its consumers run during the tanh)
GATE_MC = {"r": (0, 1), "z": (2, 3), "n": (4, 5)}


def build_program(T, debug=False, enable_asserts=False):
    """Build + compile the per-core Bass program (same program on all cores)."""
    nc = bacc.Bacc(
        "TRN2",
        debug=debug,
        enable_asserts=enable_asserts,
        target_bir_lowering=False,
        num_devices=NCORES,
    )

    SL = 2 * BL  # 16 columns per h slot: [kc0 b0..7 | kc1 b0..7]

    # DRAM inputs (already in final on-chip (partition, free) layout, host-prepped)
    w_dram = nc.dram_tensor("w_tiles", (128, 12 * 128), BF16, kind="ExternalInput")
    cr_dram = nc.dram_tensor("cr_stat", (SL, 128), BF16, kind="ExternalInput")
    cz_dram = nc.dram_tensor("cz_stat", (SL, 128), BF16, kind="ExternalInput")
    cn_dram = nc.dram_tensor("cn_stat", (SL, 128), BF16, kind="ExternalInput")
    i16_dram = nc.dram_tensor("ident16", (SL, SL), BF16, kind="ExternalInput")
    cin_dram = nc.dram_tensor("cin_n", (128, SL), F32, kind="ExternalInput")
    wout_dram = nc.dram_tensor("wout_t", (128, 2 * OUT_D), F32, kind="ExternalInput")
    ones_dram = nc.dram_tensor("ones1", (1, OUT_D), F32, kind="ExternalInput")
    bout_dram = nc.dram_tensor("bout_row", (1, OUT_D), F32, kind="ExternalInput")
    out_dram = nc.dram_tensor("out", (BL, T, OUT_D), F32, kind="ExternalOutput")

    with tile.TileContext(nc) as tc, ExitStack() as ctx:
        const = ctx.enter_context(tc.tile_pool(name="const", bufs=1))
        hsbuf = ctx.enter_context(tc.tile_pool(name="hsbuf", bufs=1))
        work = ctx.enter_context(tc.tile_pool(name="work", bufs=3))
        pr_pool = ctx.enter_context(tc.tile_pool(name="prp", bufs=2, space="PSUM"))
        pn_pool = ctx.enter_context(tc.tile_pool(name="pnp", bufs=2, space="PSUM"))
        pz_pool = ctx.enter_context(tc.tile_pool(name="pzp", bufs=2, space="PSUM"))
        pout_pool = ctx.enter_context(tc.tile_pool(name="poutp", bufs=2, space="PSUM"))

        wsb = const.tile([128, 12 * 128], BF16)
        crs = const.tile([SL, 128], BF16)
        czs = const.tile([SL, 128], BF16)
        cns = const.tile([SL, 128], BF16)
        i16 = const.tile([SL, SL], BF16)
        cin = const.tile([128, SL], F32)
        wout = const.tile([128, 2 * OUT_D], F32)
        ones1 = const.tile([1, OUT_D], F32)
        boutr = const.tile([1, OUT_D], F32)

        nc.sync.dma_start(wsb[:], w_dram[:])
        nc.sync.dma_start(crs[:], cr_dram[:])
        nc.sync.dma_start(czs[:], cz_dram[:])
        nc.sync.dma_start(cns[:], cn_dram[:])
        nc.sync.dma_start(i16[:], i16_dram[:])
        nc.sync.dma_start(cin[:], cin_dram[:])
        nc.sync.dma_start(wout[:], wout_dram[:])
        nc.sync.dma_start(ones1[:], ones_dram[:])
        nc.sync.dma_start(boutr[:], bout_dram[:])

        # fp32 hidden-state ring: slot s holds h after step s-1 (slot 0 = zeros)
        hs = hsbuf.tile([128, (T + 1) * SL], F32)
        nc.vector.memset(hs[:, 0:SL], 0.0)

        # fp16 cast of h for the matmul moving operand
        hbf = work.tile([128, SL], BF16, tag="hbf")
        nc.vector.memset(hbf[:], 0.0)

        def wtile(kc, mc):
            return wsb[:, (kc * 6 + mc) * 128 : (kc * 6 + mc + 1) * 128]

        def gate_mms(psum, gate, hbf):
            mcs = GATE_MC[gate]
            for i, mc in enumerate(mcs):
                for kc in range(2):
                    nc.tensor.matmul(
                        psum[:, i * BL : (i + 1) * BL],
                        wtile(kc, mc),
                        hbf[:, kc * BL : (kc + 1) * BL],
                        start=False,
                        stop=(i == 1 and kc == 1),
                        skip_group_check=True,
                    )

        from concourse.alu_op_type import AluOpType

        for t in range(T):
            hin = hs[:, t * SL : (t + 1) * SL]
            hout = hs[:, (t + 1) * SL : (t + 2) * SL]

            pr = pr_pool.tile([128, SL], F32)
            pn = pn_pool.tile([128, SL], F32)
            pz = pz_pool.tile([128, SL], F32)

            # bias seeds first: they don't depend on h, so the PE runs them
            # during the previous step's elementwise tail
            nc.tensor.matmul(pr[:], crs[:], i16[:], start=True, stop=True)
            nc.tensor.matmul(pn[:], cns[:], i16[:], start=True, stop=True)
            nc.tensor.matmul(pz[:], czs[:], i16[:], start=True, stop=True)
            gate_mms(pr, "r", hbf)
            gate_mms(pn, "n", hbf)
            gate_mms(pz, "z", hbf)

            sr = work.tile([128, SL], F32, tag="sr")
            sz = work.tile([128, SL], F32, tag="sz")
            t1 = work.tile([128, SL], F32, tag="t1")
            t2 = work.tile([128, SL], F32, tag="t2")
            nt = work.tile([128, SL], F32, tag="nt")
            zh = work.tile([128, SL], F32, tag="zh")
            mneg = work.tile([128, SL], F32, tag="mneg")

            # ACT queue order: sigmoid(r), sigmoid(z), tanh
            nc.scalar.activation(sr[:], pr[:], AF.Sigmoid)
            nc.scalar.activation(sz[:], pz[:], AF.Sigmoid)
            # n = tanh(i_n + r * (gh_n + bhh_n))
            nc.vector.tensor_mul(t1[:], sr[:], pn[:])
            nc.vector.tensor_add(t2[:], t1[:], cin[:])
            nc.scalar.activation(nt[:], t2[:], AF.Tanh)
            # keep-warm: tiny PE op pinned mid-gap via the t2 dependency
            dum = pout_pool.tile([1, 1], F32, tag="ps")
            nc.tensor.matmul(dum[0:1, 0:1], ones1[0:1, 0:1], t2[0:1, 0:1], start=True, stop=True)
            # h' = (1-z)*n + z*h  via  mneg = (z-1)*n,  h' = z*h - mneg
            nc.vector.tensor_mul(zh[:], sz[:], hin[:])
            nc.vector.scalar_tensor_tensor(
                mneg[:], sz[:], 1.0, nt[:], AluOpType.subtract, AluOpType.mult
            )
            hbf = work.tile([128, SL], BF16, tag="hbf")
            nc.vector.tensor_sub(hbf[:], zh[:], mneg[:])
            nc.vector.tensor_sub(hout[:], zh[:], mneg[:])

        # ---- projection: out[b, t, :] = hs[b, t] @ Wout.T + bout ----
        hs3 = hs[:].rearrange("p (s c) -> p s c", c=SL)
        t0 = 0
        while t0 < T:
            csz = min(PROJ_CHUNK, T - t0)
            mm = csz * BL
            ps = pout_pool.tile([mm, OUT_D], F32, tag="ps")
            nc.tensor.matmul(ps[:], ones1[:, 0:mm], boutr[:], start=True, stop=True)
            for kc in range(2):
                stg = work.tile([128, mm], F32, tag=f"stgl{kc}")
                nc.vector.tensor_copy(
                    stg[:], hs3[:, t0 + 1 : t0 + 1 + csz, kc * BL : (kc + 1) * BL]
                )
                nc.tensor.matmul(
                    ps[:],
                    stg[:],
                    wout[:, kc * OUT_D : (kc + 1) * OUT_D],
                    start=False,
                    stop=(kc == 1),
                    skip_group_check=True,
                )
            stage = work.tile([mm, OUT_D], F32, tag="stage")
            nc.scalar.copy(stage[:], ps[:])
            dst = out_dram.rearrange("b t d -> t b d")[t0 : t0 + csz, :, :]
            nc.sync.dma_start(dst, stage[:])
            t0 += csz

    nc.compile()
    return nc


def host_prep(z, Wih, bih, Whh, bhh, Wout, bout, T):
    """Numpy preprocessing into per-core on-chip layouts."""
    z = np.asarray(z, np.float32)
    gi = z @ np.asarray(Wih, np.float32).T + np.asarray(bih, np.float32)  # (B, 768)
    bhh = np.asarray(bhh, np.float32)
    WhhT = np.ascontiguousarray(np.asarray(Whh, np.float32).T)  # (256, 768)
    # stationary weight tiles: wsb[k, (kc*6+mc)*128+j] = WhhT[kc*128+k, mc*128+j]
    wsb = (
        WhhT.reshape(2, 128, 6, 128)
        .transpose(1, 0, 2, 3)
        .reshape(128, 12 * 128)
        .astype(np.float16)
    )
    WoutT = np.asarray(Wout, np.float32).T  # (256, 128)
    wout_t = np.ascontiguousarray(
        WoutT.reshape(2, 128, OUT_D).transpose(1, 0, 2).reshape(128, 2 * OUT_D)
    ).astype(np.float32)
    i16 = np.eye(2 * BL, dtype=np.float16)
    ones1 = np.ones((1, OUT_D), np.float32)
    bout_row = np.asarray(bout, np.float32).reshape(1, OUT_D)
    cn_stat = (
        np.repeat(bhh[512:].reshape(2, 1, 128), BL, axis=1)
        .reshape(2 * BL, 128)
        .astype(np.float16)
    )

    in_maps = []
    for c in range(NCORES):
        gic = gi[c * BL : (c + 1) * BL]  # (BL, 768)
        Crz = gic[:, :512] + bhh[:512]  # (BL, 512)
        crz_stat = Crz.reshape(BL, 4, 128).transpose(1, 0, 2).reshape(4 * BL, 128)
        cr_stat = crz_stat[0 : 2 * BL].astype(np.float16)
        cz_stat = crz_stat[2 * BL : 4 * BL].astype(np.float16)
        cin = np.ascontiguousarray(
            gic[:, 512:].reshape(BL, 2, 128).transpose(2, 1, 0).reshape(128, 2 * BL)
        ).astype(np.float32)
        in_maps.append(
            {
                "w_tiles": wsb,
                "cr_stat": cr_stat,
                "cz_stat": cz_stat,
                "cn_stat": cn_stat,
                "ident16": i16,
                "cin_n": cin,
                "wout_t": wout_t,
                "ones1": ones1,
                "bout_row": bout_row,
            }
        )
    return in_maps


_CACHED = {}


def _get_program(T):
    if T not in _CACHED:
        _CACHED[T] = build_program(T)
    return _CACHED[T]


def run(z, Wih, bih, Whh, bhh, Wout, bout, n_frames, trace=False):
    T = int(n_frames)
    nc = _get_program(T)
    in_maps = host_prep(z, Wih, bih, Whh, bhh, Wout, bout, T)
    res = bass_utils.run_bass_kernel_spmd(
        nc, in_maps, core_ids=list(range(NCORES)), trace=trace
    )
    out = np.concatenate([res.results[c]["out"] for c in range(NCORES)], axis=0)
    return out.astype(np.float32), res


def kernel(z, Wih, bih, Whh, bhh, Wout, bout, n_frames):
    try:
        out, _ = run(z, Wih, bih, Whh, bhh, Wout, bout, n_frames)
    except Exception:
        # transient device/runtime failures (e.g. core contention) — retry once
        import time as _time

        _time.sleep(5)
        out, _ = run(z, Wih, bih, Whh, bhh, Wout, bout, n_frames)
    return out


def make_runner(z, Wih, bih, Whh, bhh, Wout, bout, n_frames):
    """Build the PJRT callable once; returns (fn_exec, fn_fetch) where
    fn_exec() launches one execution (async) and returns the out handles,
    fn_fetch(outs) assembles the full (64, T, 128) fp32 output."""
    import jax
    from jax.sharding import Mesh, PartitionSpec
    from jax.experimental.shard_map import shard_map
    from concourse import bass2jax
    from concourse.bass2jax import _bass_exec_p, install_neuronx_cc_hook
    import concourse.mybir as mb

    T = int(n_frames)
    nc = _get_program(T)
    in_maps = host_prep(z, Wih, bih, Whh, bhh, Wout, bout, T)
    install_neuronx_cc_hook()

    in_names, out_names, out_avals, zero_outs = [], [], [], []
    for alloc in nc.m.functions[0].allocations:
        if not isinstance(alloc, mb.MemoryLocationSet):
            continue
        name = alloc.memorylocations[0].name
        if alloc.kind == "ExternalInput":
            if nc.partition_id_tensor is None or name != nc.partition_id_tensor.name:
                in_names.append(name)
        elif alloc.kind == "ExternalOutput":
            out_names.append(name)
            shape = tuple(alloc.tensor_shape)
            dtype = mybir.dt.np(alloc.dtype)
            out_avals.append(jax.core.ShapedArray(shape, dtype))
            zero_outs.append(np.zeros(shape, dtype))
    n_params = len(in_names)
    all_in = list(in_names) + out_names
    pname = nc.partition_id_tensor.name if nc.partition_id_tensor else None
    if pname is not None:
        all_in.append(pname)

    def _body(*args):
        operands = list(args)
        if pname is not None:
            operands.append(bass2jax.partition_id_tensor())
        return tuple(
            _bass_exec_p.bind(
                *operands,
                out_avals=tuple(out_avals),
                in_names=tuple(all_in),
                out_names=tuple(out_names),
                lowering_input_output_aliases=(),
                sim_require_finite=True,
                sim_require_nnan=True,
                nc=nc,
            )
        )

    devices = jax.devices()[:NCORES]
    mesh = Mesh(np.asarray(devices), ("core",))
    n_outs = len(out_avals)
    fn = jax.jit(
        shard_map(
            _body,
            mesh=mesh,
            in_specs=(PartitionSpec("core"),) * (n_params + n_outs),
            out_specs=(PartitionSpec("core"),) * n_outs,
            check_rep=False,
        ),
        keep_unused=True,
    )
    per_core = [[np.asarray(m[name]) for name in in_names] for m in in_maps]
    concat_in = [
        np.concatenate([per_core[c][i] for c in range(NCORES)], axis=0)
        for i in range(n_params)
    ]
    concat_zeros = [
        np.zeros((NCORES * zz.shape[0], *zz.shape[1:]), zz.dtype) for zz in zero_outs
    ]
    args_dev = [jax.device_put(a) for a in concat_in + concat_zeros]

    def fn_exec():
        return fn(*args_dev)

    def fn_fetch(outs):
        o = np.asarray(outs[0]).reshape(NCORES, *out_avals[0].shape)
        return o.reshape(B, T, OUT_D).astype(np.float32)

    return fn_exec, fn_fetch



# revision 3
# speedup vs baseline: 1.4498x; 1.4498x over previous
"""GRU decoder kernel for Trainium2 (Bass/Tile), SPMD over 8 NeuronCores.

Differences from the v1 baseline (see kernel.py docstring for the problem):
the final blend h' = z*h + (1-z)*n is folded into the recurrent matmul by
linearity: the PE accumulates W @ zh + W @ pneg where zh = z*h16 and
pneg = (1-z)*n are fed as two separate fp16 moving operands into the same
PSUM accumulation group.  This removes the h-cast (hbf) node and one
DVE->PE arrow from the loop-carried critical cycle.  The tanh oddness
identity tanh(-x) = -tanh(x) lets both movers use the SAME weight tiles:
  t1 = -r*pn, t2 = t1 - i_n, nt = tanh(t2) = -n, pneg = (z-1)*nt = (1-z)*n.
The hidden state is carried fp16 (h = zh + pneg summed off-cycle into a
fp16 ring read by the projection).  The zh-side matmul feeds run mid-step
(as soon as sigmoid(z) lands), which also keeps the PE p-state warm, so the
cycle tail only waits for the 12 pneg-side feeds.
"""

import sys

sys.path.insert(0, "/opt/trn_rl_repo")

import numpy as np
from contextlib import ExitStack

import concourse.bass as bass
import concourse.tile as tile
from concourse import bacc, mybir
from concourse import bass_utils
from concourse.alu_op_type import AluOpType

F32 = mybir.dt.float32
F16 = mybir.dt.float16
AF = mybir.ActivationFunctionType

H = 256
B = 64
NCORES = 8
BL = B // NCORES  # 8 batch rows per core
OUT_D = 128
PROJ_CHUNK = 16  # timesteps per projection matmul (16*8 batch = 128 = M)

# gate order within the sweep: r first (feeds sigmoid early), n second
# (feeds the tanh chain), z last (its consumers run during the tanh)
GATE_MC = {"r": (0, 1), "z": (2, 3), "n": (4, 5)}


def build_program(T, debug=False, enable_asserts=False):
    """Build + compile the per-core Bass program (same program on all cores)."""
    nc = bacc.Bacc(
        "TRN2",
        debug=debug,
        enable_asserts=enable_asserts,
        target_bir_lowering=False,
        num_devices=NCORES,
    )

    SL = 2 * BL  # 16 columns per h slot: [kc0 b0..7 | kc1 b0..7]

    # DRAM inputs (already in final on-chip (partition, free) layout, host-prepped)
    w_dram = nc.dram_tensor("w_tiles", (128, 12 * 128), F16, kind="ExternalInput")
    cr_dram = nc.dram_tensor("cr_stat", (SL, 128), F16, kind="ExternalInput")
    cz_dram = nc.dram_tensor("cz_stat", (SL, 128), F16, kind="ExternalInput")
    cn_dram = nc.dram_tensor("cn_stat", (SL, 128), F16, kind="ExternalInput")
    i16_dram = nc.dram_tensor("ident16", (SL, SL), F16, kind="ExternalInput")
    cin_dram = nc.dram_tensor("cin_n", (128, SL), F32, kind="ExternalInput")
    wout_dram = nc.dram_tensor("wout_t", (128, 2 * OUT_D), F32, kind="ExternalInput")
    ones_dram = nc.dram_tensor("ones1", (1, OUT_D), F32, kind="ExternalInput")
    bout_dram = nc.dram_tensor("bout_row", (1, OUT_D), F32, kind="ExternalInput")
    out_dram = nc.dram_tensor("out", (BL, T, OUT_D), F32, kind="ExternalOutput")

    with tile.TileContext(nc) as tc, ExitStack() as ctx:
        const = ctx.enter_context(tc.tile_pool(name="const", bufs=1))
        hsbuf = ctx.enter_context(tc.tile_pool(name="hsbuf", bufs=1))
        work = ctx.enter_context(tc.tile_pool(name="work", bufs=3))
        pr_pool = ctx.enter_context(tc.tile_pool(name="prp", bufs=2, space="PSUM"))
        pn_pool = ctx.enter_context(tc.tile_pool(name="pnp", bufs=2, space="PSUM"))
        pz_pool = ctx.enter_context(tc.tile_pool(name="pzp", bufs=2, space="PSUM"))
        pout_pool = ctx.enter_context(tc.tile_pool(name="poutp", bufs=2, space="PSUM"))

        wsb = const.tile([128, 12 * 128], F16)
        crs = const.tile([SL, 128], F16)
        czs = const.tile([SL, 128], F16)
        cns = const.tile([SL, 128], F16)
        i16 = const.tile([SL, SL], F16)
        cin = const.tile([128, SL], F32)
        wout = const.tile([128, 2 * OUT_D], F32)
        ones1 = const.tile([1, OUT_D], F32)
        boutr = const.tile([1, OUT_D], F32)

        nc.sync.dma_start(wsb[:], w_dram[:])
        nc.sync.dma_start(crs[:], cr_dram[:])
        nc.sync.dma_start(czs[:], cz_dram[:])
        nc.sync.dma_start(cns[:], cn_dram[:])
        nc.sync.dma_start(i16[:], i16_dram[:])
        nc.sync.dma_start(cin[:], cin_dram[:])
        nc.sync.dma_start(wout[:], wout_dram[:])
        nc.sync.dma_start(ones1[:], ones_dram[:])
        nc.sync.dma_start(boutr[:], bout_dram[:])

        # fp16 hidden-state ring: slot s holds h after step s-1 (slot 0 = zeros)
        hs = hsbuf.tile([128, (T + 1) * SL], F16)
        nc.vector.memset(hs[:, 0:SL], 0.0)

        # interleaved scan operands for the fused -(i_n + r*pn) FMA:
        #   per batch column pair (2b, 2b+1):
        #     col 2b:   d0 = 0,  d1 = w = sigmoid(-pr)   -> state = w
        #     col 2b+1: d0 = pn, d1 = -pn - cin          -> state = w*pn - pn - cin
        # and w*pn - pn - cin = -(cin + (1-w)*pn) = -(i_n + r*pn).
        d0 = hsbuf.tile([128, 2 * SL], F32)
        d1 = hsbuf.tile([128, 2 * SL], F32)
        nc.vector.memset(d0[:], 0.0)
        nc.vector.memset(d1[:], 0.0)
        d0i = d0[:].rearrange("p (b two) -> p b two", two=2)
        d1i = d1[:].rearrange("p (b two) -> p b two", two=2)

        # fp16 movers for the recurrent matmul: zh = z*h, pneg = (1-z)*n
        zh16 = work.tile([128, SL], F16, tag="zh16")
        pneg16 = work.tile([128, SL], F16, tag="pneg16")
        nc.vector.memset(zh16[:], 0.0)
        nc.vector.memset(pneg16[:], 0.0)

        def wtile(kc, mc):
            return wsb[:, (kc * 6 + mc) * 128 : (kc * 6 + mc + 1) * 128]

        def feed_all(mover, stop):
            """One accumulation feed of every gate region from `mover`."""
            for gate in ("r", "n", "z"):
                psum = {"r": pr, "n": pn, "z": pz}[gate]
                for i, mc in enumerate(GATE_MC[gate]):
                    for kc in range(2):
                        nc.tensor.matmul(
                            psum[:, i * BL : (i + 1) * BL],
                            wtile(kc, mc),
                            mover[:, kc * BL : (kc + 1) * BL],
                            start=False,
                            stop=(stop and kc == 1),
                            skip_group_check=True,
                        )

        for t in range(T):
            hin = hs[:, t * SL : (t + 1) * SL]
            hout = hs[:, (t + 1) * SL : (t + 2) * SL]

            pr = pr_pool.tile([128, SL], F32)
            pn = pn_pool.tile([128, SL], F32)
            pz = pz_pool.tile([128, SL], F32)

            # bias seeds first: they don't depend on h, so the PE runs them
            # during the previous step's elementwise tail
            nc.tensor.matmul(pr[:], crs[:], i16[:], start=True, stop=True)
            nc.tensor.matmul(pn[:], cns[:], i16[:], start=True, stop=True)
            nc.tensor.matmul(pz[:], czs[:], i16[:], start=True, stop=True)
            # zh-side feeds land mid-step (only need sigmoid(z) of step t-1);
            # they also keep the PE p-state warm through the elementwise gap.
            feed_all(zh16, stop=False)
            # pneg-side feeds are the loop-carried tail
            feed_all(pneg16, stop=True)

            sz = work.tile([128, SL], F32, tag="sz")
            sco = work.tile([128, 2 * SL], F32, tag="sco")
            nt = work.tile([128, SL], F32, tag="nt")

            # ACT queue order: w = sigmoid(-pr) (strided into d1 even cols),
            # sigmoid(z), tanh.  The d0/d1 odd-col builders run on the DVE in
            # parallel with the sigmoids (they only need pn).
            nc.scalar.activation(d1i[:, :, 0], pr[:], AF.Sigmoid, scale=-1.0)
            nc.scalar.activation(sz[:], pz[:], AF.Sigmoid)
            nc.vector.tensor_copy(d0i[:, :, 1], pn[:])
            # read pn back from d0 (SBUF) rather than PSUM: the shorter
            # write-ack keeps d1's odd cols from gating the scan
            nc.vector.scalar_tensor_tensor(
                d1i[:, :, 1],
                d0i[:, :, 1],
                -1.0,
                cin[:],
                AluOpType.mult,
                AluOpType.subtract,
            )
            # fused FMA: sco odd cols = w*pn - pn - cin = -(i_n + r*pn)
            nc.vector.tensor_tensor_scan(
                sco[:], d0[:], d1[:], 0.0, AluOpType.mult, AluOpType.add
            )
            scoi = sco[:].rearrange("p (b two) -> p b two", two=2)
            # nt = tanh(-(i_n + r*pn)) = -n
            nc.scalar.activation(nt[:], scoi[:, :, 1], AF.Tanh)

            zh16 = work.tile([128, SL], F16, tag="zh16")
            pneg16 = work.tile([128, SL], F16, tag="pneg16")
            # zh = z*h (off-cycle: runs while tanh is in flight)
            nc.vector.tensor_mul(zh16[:], sz[:], hin[:])
            # pneg = (z-1)*nt = (1-z)*n  -- the only post-tanh op on the cycle
            nc.vector.scalar_tensor_tensor(
                pneg16[:], sz[:], 1.0, nt[:], AluOpType.subtract, AluOpType.mult
            )
            # h' = zh + pneg, stored fp16 for the projection (off-cycle)
            nc.vector.tensor_add(hout[:], zh16[:], pneg16[:])

        # ---- projection: out[b, t, :] = hs[b, t] @ Wout.T + bout ----
        hs3 = hs[:].rearrange("p (s c) -> p s c", c=SL)
        t0 = 0
        while t0 < T:
            csz = min(PROJ_CHUNK, T - t0)
            mm = csz * BL
            ps = pout_pool.tile([mm, OUT_D], F32, tag="ps")
            nc.tensor.matmul(ps[:], ones1[:, 0:mm], boutr[:], start=True, stop=True)
            for kc in range(2):
                stg = work.tile([128, mm], F32, tag=f"stgl{kc}")
                nc.vector.tensor_copy(
                    stg[:], hs3[:, t0 + 1 : t0 + 1 + csz, kc * BL : (kc + 1) * BL]
                )
                nc.tensor.matmul(
                    ps[:],
                    stg[:],
                    wout[:, kc * OUT_D : (kc + 1) * OUT_D],
                    start=False,
                    stop=(kc == 1),
                    skip_group_check=True,
                )
            stage = work.tile([mm, OUT_D], F32, tag="stage")
            nc.scalar.copy(stage[:], ps[:])
            dst = out_dram.rearrange("b t d -> t b d")[t0 : t0 + csz, :, :]
            nc.sync.dma_start(dst, stage[:])
            t0 += csz

    nc.compile()
    return nc


def host_prep(z, Wih, bih, Whh, bhh, Wout, bout, T):
    """Numpy preprocessing into per-core on-chip layouts."""
    z = np.asarray(z, np.float32)
    gi = z @ np.asarray(Wih, np.float32).T + np.asarray(bih, np.float32)  # (B, 768)
    bhh = np.asarray(bhh, np.float32)
    WhhT = np.ascontiguousarray(np.asarray(Whh, np.float32).T)  # (256, 768)
    # stationary weight tiles: wsb[k, (kc*6+mc)*128+j] = WhhT[kc*128+k, mc*128+j]
    wsb = (
        WhhT.reshape(2, 128, 6, 128)
        .transpose(1, 0, 2, 3)
        .reshape(128, 12 * 128)
        .astype(np.float16)
    )
    WoutT = np.asarray(Wout, np.float32).T  # (256, 128)
    wout_t = np.ascontiguousarray(
        WoutT.reshape(2, 128, OUT_D).transpose(1, 0, 2).reshape(128, 2 * OUT_D)
    ).astype(np.float32)
    i16 = np.eye(2 * BL, dtype=np.float16)
    ones1 = np.ones((1, OUT_D), np.float32)
    bout_row = np.asarray(bout, np.float32).reshape(1, OUT_D)
    cn_stat = (
        np.repeat(bhh[512:].reshape(2, 1, 128), BL, axis=1)
        .reshape(2 * BL, 128)
        .astype(np.float16)
    )

    in_maps = []
    for c in range(NCORES):
        gic = gi[c * BL : (c + 1) * BL]  # (BL, 768)
        Crz = gic[:, :512] + bhh[:512]  # (BL, 512)
        crz_stat = Crz.reshape(BL, 4, 128).transpose(1, 0, 2).reshape(4 * BL, 128)
        cr_stat = crz_stat[0 : 2 * BL].astype(np.float16)
        cz_stat = crz_stat[2 * BL : 4 * BL].astype(np.float16)
        cin = np.ascontiguousarray(
            gic[:, 512:].reshape(BL, 2, 128).transpose(2, 1, 0).reshape(128, 2 * BL)
        ).astype(np.float32)
        in_maps.append(
            {
                "w_tiles": wsb,
                "cr_stat": cr_stat,
                "cz_stat": cz_stat,
                "cn_stat": cn_stat,
                "ident16": i16,
                "cin_n": cin,
                "wout_t": wout_t,
                "ones1": ones1,
                "bout_row": bout_row,
            }
        )
    return in_maps


_CACHED = {}


def _get_program(T):
    if T not in _CACHED:
        _CACHED[T] = build_program(T)
    return _CACHED[T]


def run(z, Wih, bih, Whh, bhh, Wout, bout, n_frames, trace=False):
    T = int(n_frames)
    nc = _get_program(T)
    in_maps = host_prep(z, Wih, bih, Whh, bhh, Wout, bout, T)
    res = bass_utils.run_bass_kernel_spmd(
        nc, in_maps, core_ids=list(range(NCORES)), trace=trace
    )
    out = np.concatenate([res.results[c]["out"] for c in range(NCORES)], axis=0)
    return out.astype(np.float32), res


def kernel(z, Wih, bih, Whh, bhh, Wout, bout, n_frames):
    try:
        out, _ = run(z, Wih, bih, Whh, bhh, Wout, bout, n_frames)
    except Exception:
        # transient device/runtime failures (e.g. core contention) — retry once
        import time as _time

        _time.sleep(5)
        out, _ = run(z, Wih, bih, Whh, bhh, Wout, bout, n_frames)
    return out


def make_runner(z, Wih, bih, Whh, bhh, Wout, bout, n_frames):
    """Build the PJRT callable once; returns (fn_exec, fn_fetch) where
    fn_exec() launches one execution (async) and returns the out handles,
    fn_fetch(outs) assembles the full (64, T, 128) fp32 output."""
    import jax
    from jax.sharding import Mesh, PartitionSpec
    from jax.experimental.shard_map import shard_map
    from concourse import bass2jax
    from concourse.bass2jax import _bass_exec_p, install_neuronx_cc_hook
    import concourse.mybir as mb

    T = int(n_frames)
    nc = _get_program(T)
    in_maps = host_prep(z, Wih, bih, Whh, bhh, Wout, bout, T)
    install_neuronx_cc_hook()

    in_names, out_names, out_avals, zero_outs = [], [], [], []
    for alloc in nc.m.functions[0].allocations:
        if not isinstance(alloc, mb.MemoryLocationSet):
            continue
        name = alloc.memorylocations[0].name
        if alloc.kind == "ExternalInput":
            if nc.partition_id_tensor is None or name != nc.partition_id_tensor.name:
                in_names.append(name)
        elif alloc.kind == "ExternalOutput":
            out_names.append(name)
            shape = tuple(alloc.tensor_shape)
            dtype = mybir.dt.np(alloc.dtype)
            out_avals.append(jax.core.ShapedArray(shape, dtype))
            zero_outs.append(np.zeros(shape, dtype))
    n_params = len(in_names)
    all_in = list(in_names) + out_names
    pname = nc.partition_id_tensor.name if nc.partition_id_tensor else None
    if pname is not None:
        all_in.append(pname)

    def _body(*args):
        operands = list(args)
        if pname is not None:
            operands.append(bass2jax.partition_id_tensor())
        return tuple(
            _bass_exec_p.bind(
                *operands,
                out_avals=tuple(out_avals),
                in_names=tuple(all_in),
                out_names=tuple(out_names),
                lowering_input_output_aliases=(),
                sim_require_finite=True,
                sim_require_nnan=True,
                nc=nc,
            )
        )

    devices = jax.devices()[:NCORES]
    mesh = Mesh(np.asarray(devices), ("core",))
    n_outs = len(out_avals)
    fn = jax.jit(
        shard_map(
            _body,
            mesh=mesh,
            in_specs=(PartitionSpec("core"),) * (n_params + n_outs),
            out_specs=(PartitionSpec("core"),) * n_outs,
            check_rep=False,
        ),
        keep_unused=True,
    )
    per_core = [[np.asarray(m[name]) for name in in_names] for m in in_maps]
    concat_in = [
        np.concatenate([per_core[c][i] for c in range(NCORES)], axis=0)
        for i in range(n_params)
    ]
    concat_zeros = [
        np.zeros((NCORES * zz.shape[0], *zz.shape[1:]), zz.dtype) for zz in zero_outs
    ]
    args_dev = [jax.device_put(a) for a in concat_in + concat_zeros]

    def fn_exec():
        return fn(*args_dev)

    def fn_fetch(outs):
        o = np.asarray(outs[0]).reshape(NCORES, *out_avals[0].shape)
        return o.reshape(B, T, OUT_D).astype(np.float32)

    return fn_exec, fn_fetch

